# revision 12
# baseline (speedup 1.0000x reference)
"""Stereo correlation cost volume kernel for Trainium2 (8 NeuronCores).

  out[b, d, h, w] = mean_c( L[b,c,h,w] * R[b,c,h,w-d] )  for w >= d, else 0
  B=8, C=64, H=128, W=256, D=64.

Sharding: data-parallel over batch; core b handles batch b.

Only a 64-wide diagonal band of the Gram G[u, w] = sum_c R[c,u] L[c,w]
is ever needed (u = w - d, d in [0,64)), so we compute four 64-row x
128-col band windows per h row:
  blk k: u in [64k, 64k+64), w in [64k, 64k+128)
(blk 3's w >= 256 half multiplies into R columns instead -> harmless
finite garbage that only ever lands in the output padding, sliced off
by the host.)

Per-core algorithm (per h row, pipelined over groups of NH=8 rows):
  1. PE computes blk0/blk1 into one 128-partition PSUM tile (blk1 at
     partition offset 64) and blk2/blk3 into a second; DVE/ACT copy the
     pair tiles to SBUF as f16, h-major: g8[p, k, h, j].
  2. Two DMAs (sync queue) write DRAM scratch rows r = p%64 at
     addr = blk_base + r*1024 + h*128 + j  (p-linear, 2 KB runs).
  3. Four skew-read DMAs (ACT queue, so their wait on the band write
     cannot stall the sync ring) re-read each blk through a
     1025-strided view: addr = blk_base + r*1025 + h*128 + d == row r,
     col r+d, i.e. the diagonal tile T[u, d] = G[u, u+d] with
     contiguous 64-elem d-runs.
  4. Next group: PE transposes T -> PT[d, u], DVE/ACT scale by 1/C
     into an f16 row tile s8[d, h, u'], and one DMA (sync) writes
     out[d, h, w = u + d] (partition stride H*WP+1).
The next group's input is prefetched at the top of each window (sync),
so the sync ring is never blocked behind a dependent DMA.

The output DRAM tensor is f16 padded to WP=320 columns so the fixed
256-wide skewed writes spill harmlessly for w >= 256; the host slices
w < 256 and casts to f32. The runner pre-zeros output buffers, so the
w < d triangle stays zero.
"""

import os
import sys

import numpy as np

sys.path.insert(0, "/opt/trn_rl_repo")

import ml_dtypes  # noqa: E402

import concourse.bass as bass  # noqa: E402
import concourse.bacc as bacc  # noqa: E402
import concourse.mybir as mybir  # noqa: E402
from concourse.bass import AP  # noqa: E402
from concourse.bass_utils import run_bass_kernel_spmd  # noqa: E402
from concourse.masks import make_identity  # noqa: E402
from concourse.tile import TileContext  # noqa: E402

B, C, H, W = 8, 64, 128, 256
D = 64
WP = 320  # padded output width
NH = 8  # h rows per group
NG = H // NH  # 16 groups
F32 = mybir.dt.float32
F16 = mybir.dt.float16

# compute/in/scratch dtype: "bf16" (fast) or "f32" (exact-ish)
USE_BF16 = os.environ.get("CORVOL_F32", "") != "1"

# scratch layout (elements of the scratch dtype), per group:
#   4 band blocks, each 64 rows x (NH*128) cols row-major
GW = NH * 128  # 1024: scratch row width (h-major within a block row)
BLK = 64 * GW  # 65536 elements per band block
GRP = 4 * BLK  # 262144 elements per group
SCR_SIZE = (NG // 2) * GRP  # two scratch tensors alternate by group parity

_CACHE = {}


def build():
    in_dt = F16 if USE_BF16 else F32
    nc = bacc.Bacc()
    lr_dram = nc.dram_tensor("lr", [C, H, 2, W], in_dt, kind="ExternalInput")
    out_dram = nc.dram_tensor("out", [D, H, WP], in_dt, kind="ExternalOutput")
    scr = [
        nc.dram_tensor(f"scratch{i}", [SCR_SIZE], in_dt, kind="Internal")
        for i in range(2)
    ]

    with TileContext(nc) as tc:
        with (
            tc.tile_pool(name="const", bufs=1) as pconst,
            tc.tile_pool(name="inp", bufs=2) as pin,
            tc.tile_pool(name="gband", bufs=3) as pg,
            tc.tile_pool(name="skew", bufs=3) as pt3,
            tc.tile_pool(name="outs", bufs=3) as ps8,
            tc.tile_pool(name="psA", bufs=2, space="PSUM") as ppa,
            tc.tile_pool(name="psB", bufs=2, space="PSUM") as ppb,
            tc.tile_pool(name="psTA", bufs=2, space="PSUM") as ppta,
            tc.tile_pool(name="psTB", bufs=2, space="PSUM") as pptb,
        ):
            ident = pconst.tile([128, 128], in_dt)
            make_identity(nc, ident)
            # warmup: absorb the gpsimd ident-write wait on PE once
            scrap0 = ppa.tile([64, 64], in_dt, tag="g2a")
            nc.tensor.transpose(
                scrap0[0:1, :], ident[0:64, 0:1], ident[0:64, 0:64]
            )

            def load_group(g):
                lr8 = pin.tile([C, NH * 2 * W], in_dt, tag="lr8")
                lr8v = lr8.rearrange("p (h t w) -> p h t w", h=NH, t=2)
                lr8f = lr8.rearrange("p (h x) -> p h x", h=NH)
                h0 = g * NH
                nc.sync.dma_start(out=lr8v, in_=lr_dram[:, h0 : h0 + NH, :, :])
                return lr8v, lr8f

            pend = {}
            for g in range(NG):
                sbase = (g // 2) * GRP
                st = scr[g % 2]

                # this group's input load [sync]
                lr8v, lr8f = load_group(g)

                # consume compute of the group from TWO windows ago, so
                # the transposes never wait on an in-flight skew read
                # (PE/DVE/ACT run these first in this window)
                s8avp = None
                if g - 2 in pend:
                    t01p, t23p = pend.pop(g - 2)
                    s8 = ps8.tile([64, NH * 256], in_dt, tag="s8")
                    s8v = s8.rearrange("p (h u) -> p h u", h=NH)
                    for hh in range(NH):
                        pta = ppta.tile([64, 128], in_dt, tag="pta")
                        ptb = pptb.tile([64, 128], in_dt, tag="ptb")
                        nc.tensor.transpose(pta, t01p[:, hh, :], ident)
                        nc.tensor.transpose(ptb, t23p[:, hh, :], ident)
                        nc.vector.tensor_scalar_mul(
                            s8v[:, hh, 0:128], pta, 1.0 / C
                        )
                        nc.scalar.mul(s8v[:, hh, 128:256], ptb, 1.0 / C)
                    s8avp = s8v

                # produce band blocks for this group
                #   blk0: u [0,64)    w [0,128)     -> pA parts 0:64
                #   blk1: u [64,128)  w [64,192)    -> pA parts 64:128
                #   blk2: u [128,192) w [128,256)   -> pB parts 0:64
                #   blk3: u [192,256) w [192,320)   -> pB parts 64:128
                # lr8f per h: cols [0,256) = L, [256,512) = R; blk3's rhs
                # spills into R cols -> finite garbage, lands in out pad.
                g8 = pg.tile([128, 2 * NH * 128], in_dt, tag="g8")
                g8v = g8.rearrange("p (k h j) -> p k h j", k=2, h=NH)
                for hh in range(NH):
                    pA = ppa.tile([128, 128], F32, tag="g2a")
                    pB = ppb.tile([128, 128], F32, tag="g2b")
                    nc.tensor.matmul(
                        pA[0:64, :],
                        lhsT=lr8v[:, hh, 1, 0:64],
                        rhs=lr8f[:, hh, 0:128],
                    )
                    nc.tensor.matmul(
                        pA[64:128, :],
                        lhsT=lr8v[:, hh, 1, 64:128],
                        rhs=lr8f[:, hh, 64:192],
                    )
                    nc.tensor.matmul(
                        pB[0:64, :],
                        lhsT=lr8v[:, hh, 1, 128:192],
                        rhs=lr8f[:, hh, 128:256],
                    )
                    nc.tensor.matmul(
                        pB[64:128, :],
                        lhsT=lr8v[:, hh, 1, 192:256],
                        rhs=lr8f[:, hh, 192:320],
                    )
                    nc.vector.tensor_copy(g8v[:, 0, hh, :], pA)
                    nc.scalar.copy(g8v[:, 1, hh, :], pB)

                # out-DMA of the consumed group [sync, before band writes]
                if s8avp is not None:
                    ph0 = (g - 2) * NH
                    nc.sync.dma_start(
                        out=AP(
                            out_dram,
                            ph0 * WP,
                            [[H * WP + 1, 64], [WP, NH], [1, 256]],
                        ),
                        in_=s8avp,
                    )

                # skewed re-read for the PREVIOUS group [sync]; its band
                # write completed during the last window, so this ring
                # entry never stalls, and nothing queues behind it
                # except band(g) below (whose copies are done by then).
                def skew_read(gg):
                    sb = (gg // 2) * GRP
                    stt = scr[gg % 2]
                    t01 = pt3.tile([128, NH * 64], in_dt, tag="t01")
                    t23 = pt3.tile([128, NH * 64], in_dt, tag="t23")
                    t01v = t01.rearrange("p (h d) -> p h d", h=NH)
                    t23v = t23.rearrange("p (h d) -> p h d", h=NH)
                    for k in range(4):
                        dest = (t01v, t23v)[k // 2]
                        pr = (k % 2) * 64
                        nc.sync.dma_start(
                            out=dest[pr : pr + 64, :, :],
                            in_=AP(
                                stt,
                                sb + k * BLK,
                                [[GW + 1, 64], [128, NH], [1, 64]],
                            ),
                        )
                    return (t01v, t23v)

                if g - 1 >= 0:
                    pend[g - 1] = skew_read(g - 1)

                # band blocks -> scratch [sync]; partition p of pair
                # tile k maps to blk (2k + p//64), row p%64 (p-linear).
                for k in range(2):
                    nc.sync.dma_start(
                        out=AP(
                            st,
                            sbase + k * 2 * BLK,
                            [[GW, 128], [128, NH], [1, 128]],
                        ),
                        in_=g8v[:, k, :, :],
                    )

            # issue the final group's skew read, then drain the last two
            pend[NG - 1] = skew_read(NG - 1)
            for gd in (NG - 2, NG - 1):
                t01p, t23p = pend.pop(gd)
                s8 = ps8.tile([64, NH * 256], in_dt, tag="s8")
                s8v = s8.rearrange("p (h u) -> p h u", h=NH)
                for hh in range(NH):
                    pta = ppta.tile([64, 128], in_dt, tag="pta")
                    ptb = pptb.tile([64, 128], in_dt, tag="ptb")
                    nc.tensor.transpose(pta, t01p[:, hh, :], ident)
                    nc.tensor.transpose(ptb, t23p[:, hh, :], ident)
                    nc.vector.tensor_scalar_mul(
                        s8v[:, hh, 0:128], pta, 1.0 / C
                    )
                    nc.scalar.mul(s8v[:, hh, 128:256], ptb, 1.0 / C)
                nc.sync.dma_start(
                    out=AP(
                        out_dram,
                        gd * NH * WP,
                        [[H * WP + 1, 64], [WP, NH], [1, 256]],
                    ),
                    in_=s8v,
                )
    nc.finalize()
    return nc


def kernel(left_feature, right_feature, max_disp):
    assert int(max_disp) == D
    left = np.asarray(left_feature, dtype=np.float32)
    right = np.asarray(right_feature, dtype=np.float32)
    assert left.shape == (B, C, H, W) and right.shape == (B, C, H, W)

    if "nc" not in _CACHE:
        _CACHE["nc"] = build()
    nc = _CACHE["nc"]

    np_dt = np.float16 if USE_BF16 else np.float32
    in_maps = []
    for b in range(B):
        lr = np.ascontiguousarray(
            np.stack([left[b], right[b]], axis=2).astype(np_dt)
        )  # [C, H, 2, W]
        in_maps.append({"lr": lr})
    res = run_bass_kernel_spmd(nc, in_maps, list(range(B)))
    _CACHE["last_results"] = res
    out = np.stack(
        [res.results[b]["out"][:, :, :W] for b in range(B)], axis=0
    )
    return out.astype(np.float32)
